# revision 1
# baseline (speedup 1.0000x reference)
"""BiLSTM-CRF negative-log-likelihood kernel for Trainium2 (8 NeuronCores).

Strategy: data-parallel over batch (16 sequences per core), params replicated.
Device computes, per core: the masked emission-score sum (the part of the CRF
numerator that needs emissions) and the CRF partition-function sum (the
denominators).  All label-indexed scalar lookups (start/end/transition scores,
output biases) are tiny and done on host in numpy.
loss = sum_b denom_b - sum_b num_b.

Device pipeline per core (B_local=16):
  P0: embedding gather (indirect DMA, 128 rows each) + PE transpose -> x^T bf16
  P1: fwd+bwd LSTM scans (independent chains; bf16 matmuls, psum bias init via
      identity matmul, Tanh/Sigmoid on ACT, 3 fused DVE ops per step)
  P2: bulk emissions matmul [9, S*16]; fused emission-tag reduction against a
      host-built (one-hot * mask) tensor; expem = Exp(em + b_out)
  P3: CRF forward scan in exponential space, tags on partitions: per step one
      9x9 matmul (exp(trans) stationary) + one DVE multiply writing into a
      column history vbuf; periodic renormalisation via PE-transpose dance,
      log corrections recorded in mbuf.
  P4: end-state extraction (indicator multiply + strided reduce), Ln, partial
      sums DMA'd out.
"""

import numpy as np
import ml_dtypes

import concourse.bass as bass
import concourse.bacc as bacc
import concourse.tile as tile
from concourse import mybir
from concourse import bass_utils

F32 = mybir.dt.float32
BF16 = mybir.dt.bfloat16
I32 = mybir.dt.int32

VOCAB, EMB, HID, L = 100000, 128, 256, 9
H = HID // 2  # 128 per direction
B_FULL, S_FULL = 128, 512
N_CORES_FULL = 8
PAD = 0

ALU = mybir.AluOpType
ACTF = mybir.ActivationFunctionType
AXL = mybir.AxisListType


def build_nc(S=S_FULL, BL=16, RN=8, phases=4):
    """Build the per-core Bass program (same program on every core).
    phases < 4 truncates the pipeline (debugging aid); the out tensor is
    then filled with probe values instead of the real result."""
    assert BL == 16
    NTOK = S * BL                 # tokens per core
    NG = NTOK // 128              # gather groups of 128 tokens
    assert NTOK % 128 == 0
    NCH = NTOK // 512             # emission chunks of 512 cols
    assert NTOK % 512 == 0
    NEV = S // RN                 # renorm events
    assert S % RN == 0

    nc = bacc.Bacc("TRN2", target_bir_lowering=False, debug=False)

    # ---- DRAM I/O ----
    d_emb = nc.dram_tensor("emb", [VOCAB, EMB], F32, kind="ExternalInput")
    d_idx = nc.dram_tensor("idx", [128, NG], I32, kind="ExternalInput")
    d_wih = {d: nc.dram_tensor(f"wihT_{d}", [EMB, 4 * H], BF16,
                               kind="ExternalInput") for d in "fb"}
    d_whh = {d: nc.dram_tensor(f"whhT_{d}", [H, 4 * H], BF16,
                               kind="ExternalInput") for d in "fb"}
    d_bias = {d: nc.dram_tensor(f"biasT_{d}", [128, 64], BF16,
                                kind="ExternalInput") for d in "fb"}
    d_wout = {d: nc.dram_tensor(f"woutT_{d}", [H, L], BF16,
                                kind="ExternalInput") for d in "fb"}
    d_idf = nc.dram_tensor("ident_f32", [128, 128], F32, kind="ExternalInput")
    d_idb = nc.dram_tensor("ident_bf16", [128, 128], BF16,
                           kind="ExternalInput")
    d_te = nc.dram_tensor("te9", [L, L], F32, kind="ExternalInput")
    d_estart = nc.dram_tensor("expstart", [L, 1], F32, kind="ExternalInput")
    d_eend = nc.dram_tensor("expend", [L, 1], F32, kind="ExternalInput")
    d_bout = nc.dram_tensor("bout9", [L, 1], F32, kind="ExternalInput")
    d_ones9 = nc.dram_tensor("ones9", [L, 1], F32, kind="ExternalInput")
    d_ones16 = nc.dram_tensor("ones16", [16, 1], F32, kind="ExternalInput")
    d_ohm = nc.dram_tensor("ohm", [L, NTOK], F32, kind="ExternalInput")
    d_indrep = nc.dram_tensor("indrep", [L, NTOK], F32, kind="ExternalInput")
    d_indE = nc.dram_tensor("indE", [16, NEV], F32, kind="ExternalInput")
    d_out = nc.dram_tensor("out2", [1, 2], F32, kind="ExternalOutput")

    with tile.TileContext(nc) as tc:
        persist = tc.alloc_tile_pool(name="persist", bufs=1)

        # ---- persistent small tensors ----
        idx_t = persist.tile([128, NG], I32, name="idx_t")
        nc.sync.dma_start(idx_t[:], d_idx[:])
        wih, whh, biasT, wout = {}, {}, {}, {}
        for d in "fb":
            wih[d] = persist.tile([EMB, 4 * H], BF16, name=f"wih_{d}")
            nc.sync.dma_start(wih[d][:], d_wih[d][:])
            whh[d] = persist.tile([H, 4 * H], BF16, name=f"whh_{d}")
            nc.sync.dma_start(whh[d][:], d_whh[d][:])
            biasT[d] = persist.tile([128, 64], BF16, name=f"biasT_{d}")
            nc.sync.dma_start(biasT[d][:], d_bias[d][:])
            wout[d] = persist.tile([H, L], BF16, name=f"wout_{d}")
            nc.sync.dma_start(wout[d][:], d_wout[d][:])
        idf = persist.tile([128, 128], F32, name="idf")
        nc.sync.dma_start(idf[:], d_idf[:])
        idb = persist.tile([128, 128], BF16, name="idb")
        nc.sync.dma_start(idb[:], d_idb[:])
        te9 = persist.tile([L, L], F32, name="te9_t")
        nc.sync.dma_start(te9[:], d_te[:])
        estart = persist.tile([L, 1], F32, name="estart_t")
        nc.sync.dma_start(estart[:], d_estart[:])
        eend = persist.tile([L, 1], F32, name="eend_t")
        nc.sync.dma_start(eend[:], d_eend[:])
        bout = persist.tile([L, 1], F32, name="bout_t")
        nc.sync.dma_start(bout[:], d_bout[:])
        ones9 = persist.tile([L, 1], F32, name="ones9_t")
        nc.sync.dma_start(ones9[:], d_ones9[:])
        ones16 = persist.tile([16, 1], F32, name="ones16_t")
        nc.sync.dma_start(ones16[:], d_ones16[:])
        mbuf = persist.tile([16, NEV], F32, name="mbuf")
        indE = persist.tile([16, NEV], F32, name="indE_t")
        nc.sync.dma_start(indE[:], d_indE[:])
        emacc = persist.tile([L, NCH], F32, name="emacc")
        out_sb = persist.tile([1, 2], F32, name="out_sb")

        pool_h = tc.alloc_tile_pool(name="hpool", bufs=1, side="right")
        hbuf = {d: pool_h.tile([H, NTOK], BF16, name=f"hbuf_{d}")
                for d in "fb"}

        # ================= Phase 0: embedding gather + transpose ============
        pool_x = tc.alloc_tile_pool(name="xpool", bufs=1)
        xT = pool_x.tile([128, NTOK], BF16, name="xT")
        pool_g = tc.alloc_tile_pool(name="gpool", bufs=4)
        pool_gp = tc.alloc_tile_pool(name="gppool", bufs=2, space="PSUM")

        # interleave gather order so both scan directions can start early
        g_order = []
        lo, hi = 0, NG - 1
        while lo <= hi:
            g_order.append(lo)
            if hi != lo:
                g_order.append(hi)
            lo += 1
            hi -= 1
        for g in g_order:
            stage = pool_g.tile([128, EMB], F32, name="stage", tag="stage")
            nc.gpsimd.indirect_dma_start(
                out=stage[:],
                out_offset=None,
                in_=d_emb[:],
                in_offset=bass.IndirectOffsetOnAxis(ap=idx_t[:, g:g + 1],
                                                    axis=0),
            )
            tp = pool_gp.tile([128, 128], F32, name="tp", tag="tp")
            nc.tensor.transpose(out=tp[:], in_=stage[:], identity=idf[:])
            nc.vector.tensor_copy(out=xT[:, 128 * g:128 * (g + 1)], in_=tp[:])

        # ================= Phase 1: dual LSTM scan ==========================
        pool_s1 = tc.alloc_tile_pool(name="scan", bufs=1)
        pool_rot = tc.alloc_tile_pool(name="scanrot", bufs=3)
        pool_ps = tc.alloc_tile_pool(name="scanps", bufs=2, space="PSUM")

        zero16 = pool_s1.tile([H, 16], BF16, name="zero16")
        nc.vector.memset(zero16[:], 0.0)
        A = {}
        for d in "fb":
            A[d] = pool_s1.tile([128, 32], F32, name=f"A_{d}")
            nc.vector.memset(A[d][:, 16:32], 0.0)  # c0 = 0

        nsteps = S if phases >= 1 else 0
        for t in range(nsteps):
            for d in "fb":
                tx = t if d == "f" else S - 1 - t  # absolute time index
                txp = tx - 1 if d == "f" else tx + 1
                ps = pool_ps.tile([128, 64], F32, name=f"ps_{d}", tag=f"ps{d}")
                # bias init via identity matmul; gate block order [g,i,f,o]
                nc.tensor.matmul(out=ps[:], lhsT=idb[:], rhs=biasT[d][:],
                                 start=True, stop=False)
                for k in range(4):
                    nc.tensor.matmul(
                        out=ps[:, 16 * k:16 * (k + 1)],
                        lhsT=wih[d][:, 128 * k:128 * (k + 1)],
                        rhs=xT[:, 16 * tx:16 * tx + 16],
                        start=False, stop=False)
                hprev = (zero16[:] if t == 0
                         else hbuf[d][:, 16 * txp:16 * txp + 16])
                for k in range(4):
                    nc.tensor.matmul(
                        out=ps[:, 16 * k:16 * (k + 1)],
                        lhsT=whh[d][:, 128 * k:128 * (k + 1)],
                        rhs=hprev,
                        start=False, stop=(k == 3))
                Y = pool_rot.tile([128, 48], F32, name=f"Y_{d}", tag=f"Y{d}")
                nc.scalar.activation(A[d][:, 0:16], ps[:, 0:16], ACTF.Tanh)
                nc.scalar.activation(Y[:, 0:48], ps[:, 16:64], ACTF.Sigmoid)
                # X = [tanh(g)|c] * [Y_i|Y_f] = [p | f*c]
                X = pool_rot.tile([128, 32], F32, name=f"X_{d}", tag=f"X{d}")
                nc.vector.tensor_tensor(out=X[:], in0=A[d][:, 0:32],
                                        in1=Y[:, 0:32], op=ALU.mult)
                nc.vector.tensor_tensor(out=A[d][:, 16:32], in0=X[:, 0:16],
                                        in1=X[:, 16:32], op=ALU.add)
                tcl = pool_rot.tile([128, 16], F32, name=f"tc_{d}",
                                    tag=f"tc{d}")
                nc.scalar.activation(tcl[:], A[d][:, 16:32], ACTF.Tanh)
                nc.vector.tensor_tensor(
                    out=hbuf[d][:, 16 * tx:16 * tx + 16],
                    in0=tcl[:], in1=Y[:, 32:48], op=ALU.mult)

        pool_ps.release()
        pool_rot.release()
        pool_s1.release()
        pool_gp.release()
        pool_g.release()
        pool_x.release()

        if phases >= 2:
            # ============= Phase 2: emissions + em_tag + exp ================
            pool_em = tc.alloc_tile_pool(name="empool", bufs=1)
            expem = pool_em.tile([L, NTOK], F32, name="expem")
            pool_ohm = tc.alloc_tile_pool(name="ohmpool", bufs=1)
            ohm_t = pool_ohm.tile([L, NTOK], F32, name="ohm_t")
            nc.sync.dma_start(ohm_t[:], d_ohm[:])
            pool_er = tc.alloc_tile_pool(name="emrot", bufs=2)
            pool_eps = tc.alloc_tile_pool(name="emps", bufs=2, space="PSUM")
            for c in range(NCH):
                sl = slice(512 * c, 512 * (c + 1))
                pe = pool_eps.tile([L, 512], F32, name="pe", tag="pe")
                nc.tensor.matmul(out=pe[:], lhsT=wout["f"][:],
                                 rhs=hbuf["f"][:, sl], start=True, stop=False)
                nc.tensor.matmul(out=pe[:], lhsT=wout["b"][:],
                                 rhs=hbuf["b"][:, sl], start=False, stop=True)
                scr = pool_er.tile([L, 512], F32, name="scr", tag="scr")
                nc.vector.tensor_tensor(out=scr[:], in0=pe[:],
                                        in1=ohm_t[:, sl], op=ALU.mult)
                nc.vector.tensor_reduce(out=emacc[:, c:c + 1], in_=scr[:],
                                        axis=AXL.X, op=ALU.add)
                nc.scalar.activation(expem[:, sl], pe[:], ACTF.Exp,
                                     bias=bout[:])

            emaccs = pool_er.tile([L, 1], F32, name="emaccs", tag="emaccs")
            nc.vector.tensor_reduce(out=emaccs[:], in_=emacc[:], axis=AXL.X,
                                    op=ALU.add)
            pss = pool_eps.tile([1, 1], F32, name="pss", tag="pss")
            nc.tensor.matmul(out=pss[:], lhsT=ones9[:], rhs=emaccs[:],
                             start=True, stop=True)
            nc.vector.tensor_copy(out=out_sb[:, 0:1], in_=pss[:])

            pool_eps.release()
            pool_er.release()
            pool_ohm.release()
        else:
            nc.vector.tensor_copy(out=out_sb[:, 0:1],
                                  in_=hbuf["f"][0:1, 0:1])
            nc.vector.tensor_copy(out=out_sb[:, 1:2],
                                  in_=hbuf["b"][0:1, 0:1])
        pool_h.release()

        if phases >= 3:
            # ============= Phase 3: CRF forward scan (exp space) ============
            pool_crf = tc.alloc_tile_pool(name="crfpool", bufs=1,
                                          side="right")
            vbuf = pool_crf.tile([L, NTOK], F32, name="vbuf")
            pool_cps = tc.alloc_tile_pool(name="crfps", bufs=2, space="PSUM")
            pool_cr = tc.alloc_tile_pool(name="crfrot", bufs=2, side="right")

            # v0 = exp(start) * expem[:, 0]
            nc.vector.tensor_scalar(out=vbuf[:, 0:16], in0=expem[:, 0:16],
                                    scalar1=estart[:], scalar2=None,
                                    op0=ALU.mult)
            for t in range(1, S):
                s_ps = pool_cps.tile([L, 16], F32, name="s_ps", tag="sps")
                nc.tensor.matmul(out=s_ps[:], lhsT=te9[:],
                                 rhs=vbuf[:, 16 * (t - 1):16 * t],
                                 start=True, stop=True)
                nc.vector.tensor_tensor(
                    out=vbuf[:, 16 * t:16 * (t + 1)], in0=s_ps[:],
                    in1=expem[:, 16 * t:16 * (t + 1)], op=ALU.mult)
                if t % RN == RN - 1:
                    e = t // RN
                    col = slice(16 * t, 16 * (t + 1))
                    vt_ps = pool_cps.tile([16, L], F32, name="vt_ps",
                                          tag="vtps")
                    nc.tensor.transpose(out=vt_ps[:], in_=vbuf[:, col],
                                        identity=idf[0:L, 0:L])
                    nc.vector.tensor_reduce(out=mbuf[:, e:e + 1],
                                            in_=vt_ps[:], axis=AXL.X,
                                            op=ALU.max)
                    rt = pool_cr.tile([16, 1], F32, name="rt", tag="rt")
                    nc.vector.reciprocal(out=rt[:], in_=mbuf[:, e:e + 1])
                    vts = pool_cr.tile([16, L], F32, name="vts", tag="vts")
                    nc.vector.tensor_scalar(out=vts[:], in0=vt_ps[:],
                                            scalar1=rt[:], scalar2=None,
                                            op0=ALU.mult)
                    v2_ps = pool_cps.tile([L, 16], F32, name="v2_ps",
                                          tag="v2ps")
                    nc.tensor.transpose(out=v2_ps[:], in_=vts[:],
                                        identity=idf[0:16, 0:16])
                    nc.vector.tensor_copy(out=vbuf[:, col], in_=v2_ps[:])

            pool_cr.release()
            pool_cps.release()
            pool_em.release()

            if phases >= 4:
                # ============= Phase 4: finals ==============================
                pool_f4 = tc.alloc_tile_pool(name="f4", bufs=1)
                pool_fps = tc.alloc_tile_pool(name="f4ps", bufs=2,
                                              space="PSUM")
                indrep_t = pool_f4.tile([L, NTOK], F32, name="indrep_t")
                nc.sync.dma_start(indrep_t[:], d_indrep[:])
                prod = pool_f4.tile([L, NTOK], F32, name="prod")
                nc.vector.tensor_tensor(out=prod[:], in0=vbuf[:],
                                        in1=indrep_t[:], op=ALU.mult)
                fv = pool_f4.tile([L, 16], F32, name="fv")
                nc.vector.tensor_reduce(
                    out=fv[:],
                    in_=prod[:].rearrange("p (t b) -> p b t", b=16),
                    axis=AXL.X, op=ALU.add)
                w_ps = pool_fps.tile([1, 16], F32, name="w_ps", tag="wps")
                nc.tensor.matmul(out=w_ps[:], lhsT=eend[:], rhs=fv[:],
                                 start=True, stop=True)
                lw = pool_f4.tile([1, 16], F32, name="lw")
                nc.scalar.activation(lw[:], w_ps[:], ACTF.Ln)
                lwT_ps = pool_fps.tile([16, 1], F32, name="lwT_ps",
                                       tag="lwT")
                nc.tensor.transpose(out=lwT_ps[:], in_=lw[:],
                                    identity=idf[0:1, 0:1])
                lm = pool_f4.tile([16, NEV], F32, name="lm")
                nc.scalar.activation(lm[:], mbuf[:], ACTF.Ln)
                lmscr = pool_f4.tile([16, NEV], F32, name="lmscr")
                lct = pool_f4.tile([16, 1], F32, name="lct")
                nc.vector.tensor_tensor(out=lmscr[:], in0=lm[:],
                                        in1=indE[:], op=ALU.mult)
                nc.vector.tensor_reduce(out=lct[:], in_=lmscr[:],
                                        axis=AXL.X, op=ALU.add)
                dst = pool_f4.tile([16, 1], F32, name="dst")
                nc.vector.tensor_tensor(out=dst[:], in0=lwT_ps[:],
                                        in1=lct[:], op=ALU.add)
                dtot_ps = pool_fps.tile([1, 1], F32, name="dtot_ps",
                                        tag="dtot")
                nc.tensor.matmul(out=dtot_ps[:], lhsT=ones16[:], rhs=dst[:],
                                 start=True, stop=True)
                nc.vector.tensor_copy(out=out_sb[:, 1:2], in_=dtot_ps[:])
                pool_fps.release()
                pool_f4.release()
            else:
                nc.vector.tensor_copy(out=out_sb[:, 1:2], in_=vbuf[0:1, 0:1])
            pool_crf.release()
        elif phases >= 2:
            nc.vector.tensor_copy(out=out_sb[:, 1:2], in_=expem[0:1, 0:1])
            pool_em.release()

        nc.sync.dma_start(d_out[:], out_sb[:])
        persist.release()

    nc.compile()
    return nc


# ---------------------------------------------------------------------------
# Host side
# ---------------------------------------------------------------------------

def _prep_core_inputs(core, seqs, labels, emb, w_ih, w_hh, b_ih, b_hh,
                      w_out, b_out, start_t, end_t, trans, S, BL, RN,
                      shared):
    NTOK = S * BL
    NG = NTOK // 128
    NEV = S // RN
    b0 = core * BL
    sq = seqs[b0:b0 + BL]          # [BL, S]
    lb = labels[b0:b0 + BL]
    lens = (sq != PAD).sum(axis=1).astype(np.int64)
    maskf = (sq != PAD).astype(np.float32)

    # token gather indices in (t, b) order
    toks = sq.T.reshape(-1).astype(np.int32)       # [S*BL], t-major
    idx = np.ascontiguousarray(toks.reshape(NG, 128).T)

    ohm = np.zeros((L, NTOK), np.float32)
    cols = np.arange(NTOK)
    t_of = cols // BL
    b_of = cols % BL
    ohm[lb[b_of, t_of], cols] = maskf[b_of, t_of]

    indrep = np.zeros((L, NTOK), np.float32)
    for bl in range(BL):
        indrep[:, (lens[bl] - 1) * BL + bl] = 1.0

    t_e = np.arange(NEV) * RN + RN - 1
    indE = np.ascontiguousarray(
        (t_e[None, :] <= (lens - 1)[:, None]).astype(np.float32))

    inmap = dict(shared)
    inmap["idx"] = idx
    inmap["ohm"] = ohm
    inmap["indrep"] = indrep
    inmap["indE"] = indE

    ar = np.arange(BL)
    hostnum = (start_t[lb[:, 0]]
               + (trans[lb[:, :-1], lb[:, 1:]] * maskf[:, 1:]).sum(axis=1)
               + end_t[lb[ar, lens - 1]]
               + (maskf * b_out[lb]).sum(axis=1))
    return inmap, float(hostnum.sum())


def _shared_inputs(emb, w_ih, w_hh, b_ih, b_hh, w_out, b_out, start_t,
                   end_t, trans):
    perm = [2, 0, 1, 3]  # pytorch [i,f,g,o] -> device blocks [g,i,f,o]

    def wprep(w):  # [4H, K] -> [K, 4H] col blocks in perm order
        blocks = [w[128 * p:128 * (p + 1)].T for p in perm]
        return np.ascontiguousarray(
            np.concatenate(blocks, axis=1)).astype(ml_dtypes.bfloat16)

    def bprep(bi, bh):
        bsum = (bi + bh).astype(np.float32)
        blocks = [np.repeat(bsum[128 * p:128 * (p + 1)][:, None], 16, axis=1)
                  for p in perm]
        return np.ascontiguousarray(
            np.concatenate(blocks, axis=1)).astype(ml_dtypes.bfloat16)

    shared = {
        "emb": np.ascontiguousarray(emb, dtype=np.float32),
        "ident_f32": np.eye(128, dtype=np.float32),
        "ident_bf16": np.eye(128).astype(ml_dtypes.bfloat16),
        "te9": np.ascontiguousarray(np.exp(trans.astype(np.float32))),
        "expstart": np.exp(start_t.astype(np.float32))[:, None].copy(),
        "expend": np.exp(end_t.astype(np.float32))[:, None].copy(),
        "bout9": b_out.astype(np.float32)[:, None].copy(),
        "ones9": np.ones((L, 1), np.float32),
        "ones16": np.ones((16, 1), np.float32),
    }
    for d in "fb":
        shared[f"wihT_{d}"] = wprep(w_ih[d])
        shared[f"whhT_{d}"] = wprep(w_hh[d])
        shared[f"biasT_{d}"] = bprep(b_ih[d], b_hh[d])
    shared["woutT_f"] = np.ascontiguousarray(
        w_out[:, :H].T).astype(ml_dtypes.bfloat16)
    shared["woutT_b"] = np.ascontiguousarray(
        w_out[:, H:].T).astype(ml_dtypes.bfloat16)
    return shared


_CACHE = {}


def run(inputs, S=S_FULL, BL=16, RN=8, n_cores=N_CORES_FULL, phases=4,
        **spmd_kwargs):
    seqs = np.asarray(inputs["sequences"])
    labels = np.asarray(inputs["labels"])
    emb = np.asarray(inputs["emb"], np.float32)
    w_ih = {"f": np.asarray(inputs["w_ih_f"], np.float32),
            "b": np.asarray(inputs["w_ih_b"], np.float32)}
    w_hh = {"f": np.asarray(inputs["w_hh_f"], np.float32),
            "b": np.asarray(inputs["w_hh_b"], np.float32)}
    b_ih = {"f": np.asarray(inputs["b_ih_f"], np.float32),
            "b": np.asarray(inputs["b_ih_b"], np.float32)}
    b_hh = {"f": np.asarray(inputs["b_hh_f"], np.float32),
            "b": np.asarray(inputs["b_hh_b"], np.float32)}
    w_out = np.asarray(inputs["w_out"], np.float32)
    b_out = np.asarray(inputs["b_out"], np.float32)
    start_t = np.asarray(inputs["start_t"], np.float32)
    end_t = np.asarray(inputs["end_t"], np.float32)
    trans = np.asarray(inputs["trans"], np.float32)

    key = (S, BL, RN, phases)
    if key not in _CACHE:
        _CACHE[key] = build_nc(S=S, BL=BL, RN=RN, phases=phases)
    nc = _CACHE[key]

    shared = _shared_inputs(emb, w_ih, w_hh, b_ih, b_hh, w_out, b_out,
                            start_t, end_t, trans)
    in_maps = []
    hostnum_total = 0.0
    for c in range(n_cores):
        im, hn = _prep_core_inputs(c, seqs, labels, emb, w_ih, w_hh, b_ih,
                                   b_hh, w_out, b_out, start_t, end_t, trans,
                                   S, BL, RN, shared)
        in_maps.append(im)
        hostnum_total += hn

    res = bass_utils.run_bass_kernel_spmd(nc, in_maps,
                                          core_ids=list(range(n_cores)),
                                          **spmd_kwargs)
    emtag_total = 0.0
    denom_total = 0.0
    for r in res.results:
        emtag_total += float(r["out2"][0, 0])
        denom_total += float(r["out2"][0, 1])
    loss = denom_total - (hostnum_total + emtag_total)
    return np.array(loss, dtype=np.float32), res


def kernel(**inputs):
    loss, _ = run(inputs)
    return loss

